# revision 11
# baseline (speedup 1.0000x reference)
"""EMA (exponential moving average) Trainium2 Bass kernel.

Problem: y[b,t,f] = w*x[b,t,f] + (1-w)*y[b,t-1,f], y[b,-1,:] = initial_state[b,:],
w = clip(smooth, 0, 1), x: [16, 8192, 512] f32.

Strategy (per core, batch-sharded 2 batches/core across 8 cores):
  - Chunk time into blocks of 128. Within a chunk, the scan is a lower-
    triangular matmul: P = L @ x_chunk with L[c,j] = w*(1-w)^(c-j) (c>=j).
  - The cross-chunk carry enters via a K=1 accumulated outer product:
    psum += dvec ⊗ e_k with dvec[c] = (1-w)^(c+1), e_k = previous chunk's
    last output row. L/dvec are host-precomputed runtime inputs, so the
    compiled NEFF is independent of w.
  - Output rows are produced time-REVERSED (host-side flip of L/dvec) so
    the carry row lands on PSUM partition 0 (engines can only address
    base partitions 0/32/64/96). The store DMA writes chunks as-is
    (reversed) and the host un-reverses with a cheap numpy flip.
  - Main matmul in fp32 (exact); carry matmul in float32r (fast path,
    ~1e-4 relative on a term of weight <= 1-w).
  - PSUM -> SBUF copies + carry-row extraction alternate between VectorE
    and ScalarE; DMA in/out batched 4 chunks (1 MiB) per transfer.
"""
import os
import sys
import tempfile

sys.path.insert(0, "/opt/trn_rl_repo")

import numpy as np

import concourse.bacc as bacc
import concourse.mybir as mybir
import concourse.tile as tile
from concourse import bass_utils

f32 = mybir.dt.float32
f32r = mybir.dt.float32r

N_CORES = 8
B, T, F = 16, 8192, 512
NB = B // N_CORES          # batches per core
C = 128                    # chunk length (time steps)
NCHUNK = T // C            # chunks per batch
G = 4                      # chunks per DMA group
NG = NCHUNK // G           # DMA groups per batch

_cache = {}


def _build():
    nc = bacc.Bacc("TRN2", target_bir_lowering=False, debug=False, num_devices=1)
    X = nc.dram_tensor("x", [NB, T, F], f32, kind="ExternalInput").ap()
    INIT = nc.dram_tensor("init_r", [NB, F], f32r, kind="ExternalInput").ap()
    LT = nc.dram_tensor("lt", [C, C], f32, kind="ExternalInput").ap()
    DVEC = nc.dram_tensor("dvec_r", [1, C], f32r, kind="ExternalInput").ap()
    Y = nc.dram_tensor("y", [NB, T, F], f32, kind="ExternalOutput").ap()

    with tile.TileContext(nc) as tc:
        with (
            tc.tile_pool(name="const", bufs=1) as const,
            tc.tile_pool(name="xin", bufs=4) as xin,
            tc.tile_pool(name="yout", bufs=4) as yout,
            tc.tile_pool(name="ecar", bufs=6) as ecar,
            tc.tile_pool(name="ps", bufs=8, space="PSUM") as ps,
        ):
            lt_sb = const.tile([C, C], f32)
            nc.sync.dma_start(lt_sb[:], LT)
            dvec_sb = const.tile([1, C], f32r)
            nc.sync.dma_start(dvec_sb[:], DVEC)

            e_prev = []
            for b in range(NB):
                e0 = ecar.tile([1, F], f32r, name=f"e0_{b}", tag="e")
                nc.sync.dma_start(e0[:], INIT[b : b + 1, :])
                e_prev.append(e0)

            for g in range(NG):
                for b in range(NB):
                    xt = xin.tile([C, G * F], f32, name=f"xt_{b}_{g}", tag="x")
                    src = X[b, g * G * C : (g + 1) * G * C, :].rearrange(
                        "(c p) f -> p c f", p=C
                    )
                    nc.sync.dma_start(
                        xt[:].rearrange("p (c f) -> p c f", c=G), src
                    )
                    yt = yout.tile([C, G * F], f32, name=f"yt_{b}_{g}", tag="y")
                    for c in range(G):
                        k = g * G + c
                        p = ps.tile([C, F], f32, name=f"p_{b}_{k}", tag="p")
                        nc.tensor.matmul(
                            p[:], lt_sb[:], xt[:, c * F : (c + 1) * F],
                            start=True, stop=False,
                        )
                        nc.tensor.matmul(
                            p[:], dvec_sb[:], e_prev[b][:], start=False, stop=True
                        )
                        if (k + b) % 2 == 0:
                            cp = nc.vector.tensor_copy
                        else:
                            cp = nc.scalar.copy
                        e_new = ecar.tile([1, F], f32r, name=f"e_{b}_{k}", tag="e")
                        cp(e_new[:], p[0:1, :])
                        cp(yt[:, c * F : (c + 1) * F], p[:])
                        e_prev[b] = e_new
                    dst = Y[b, g * G * C : (g + 1) * G * C, :].rearrange(
                        "(c p) f -> p c f", p=C
                    )
                    nc.sync.dma_start(
                        dst, yt[:].rearrange("p (c f) -> p c f", c=G)
                    )
    nc.compile()
    return nc


def _get_nc():
    if "nc" not in _cache:
        _cache["nc"] = _build()
    return _cache["nc"]


def _host_constants(w: float):
    # L[c, j] = w * (1-w)^(c-j) for c >= j; dvec[c] = (1-w)^(c+1).
    # Rows are emitted time-reversed (psum row c = y[t0 + C-1-c]) so both
    # are flipped along the output-row axis before transposing.
    wd = np.float64(w)
    decay = np.float64(1.0) - wd
    pows = decay ** np.arange(C + 1, dtype=np.float64)  # (1-w)^0 .. ^C
    cmj = np.arange(C)[:, None] - np.arange(C)[None, :]
    L = np.where(cmj >= 0, wd * decay ** np.clip(cmj, 0, None), 0.0)
    Lr = L[::-1, :]  # reversed output rows
    lt = np.ascontiguousarray(Lr.T).astype(np.float32)  # lhsT: [K=j, M=c]
    dvec = pows[1:][::-1].astype(np.float32).reshape(1, C)
    return lt, dvec


def _run(x, initial_state, smooth, trace=False):
    w = float(np.clip(np.float64(smooth.reshape(-1)[0]), 0.0, 1.0))
    lt, dvec = _host_constants(w)

    nc = _get_nc()
    in_maps = []
    for i in range(N_CORES):
        in_maps.append(
            {
                "x": np.ascontiguousarray(x[i * NB : (i + 1) * NB]),
                "init_r": np.ascontiguousarray(
                    initial_state[i * NB : (i + 1) * NB]
                ),
                "lt": lt,
                "dvec_r": dvec,
            }
        )
    kwargs = {}
    if trace:
        kwargs = dict(trace=True, tmpdir=tempfile.mkdtemp(prefix="ema_trace_"))
    res = bass_utils.run_bass_kernel_spmd(
        nc, in_maps, core_ids=list(range(N_CORES)), **kwargs
    )
    y = np.concatenate([res.results[i]["y"] for i in range(N_CORES)], axis=0)
    # Chunks were written time-reversed; flip each 128-row chunk back.
    y = np.ascontiguousarray(
        y.reshape(B, NCHUNK, C, F)[:, :, ::-1, :]
    ).reshape(B, T, F)
    return y, res


def kernel(x, initial_state, smooth):
    y, _ = _run(
        np.asarray(x, dtype=np.float32),
        np.asarray(initial_state, dtype=np.float32),
        np.asarray(smooth, dtype=np.float32),
    )
    return y
